# revision 63
# baseline (speedup 1.0000x reference)
"""Trainium2 Bass kernel for nn_MF2Net (two tiny MLPs + Choquet integral + softmax).

Strategy: pure data parallel over the batch dim (8 NeuronCores x 32768 rows).
x is shipped host-transposed as fp8-e4m3 [512, R] so features land on SBUF
partitions straight from DMA (no on-chip transpose, 1/4 the f32 HBM traffic).
Per core:
  - x^T DMAs all pre-issued across two queues (SP HWDGE + Pool SWDGE):
    6 supertiles of 4096 rows, then 16 per-tile chunks for a short drain
  - matmul1 per 512-row tile: H^T[128hid, 512] = DoubleRow-fp8 matmul
    (K=512 as 2 packed 256-chunks) into paired 2-bank f32 PSUM
  - ACT: H = relu(H^T + b13) per tile-pair [128, 1024] -> bf16
  - matmul2 (bf16): S[128 rows, 8] per 128-row group into a per-batch PSUM
    bank [128, 512]; lhsT = H chunk, rhs = Wcat ([W2|W4] block-diag)
  - per half-batch: DVE bias add + ACT sigmoid on [128, 256], then
    Choquet combine (both classes fused) + softmax via sigmoid(res0-res1)
Row mapping on device is purely sequential; probs/out are host-permuted to
match (cheap: 0.5 MB + 0.25 MB per core vs 64 MB for x).
"""
import numpy as np
import ml_dtypes
from contextlib import ExitStack

import concourse.bass as bass
import concourse.bacc as bacc
import concourse.tile as tile
import concourse.mybir as mybir
from concourse import bass_utils

N_CORES = 8
B = 262144
D = 512
R = B // N_CORES            # rows per core
TILE_ROWS = 512
N_TILES = R // TILE_ROWS    # 64
TILES_PER_BATCH = 16
BATCH_ROWS = TILE_ROWS * TILES_PER_BATCH   # 8192
G = BATCH_ROWS // 128                      # 64 row-groups per batch
NB = R // BATCH_ROWS                       # 4 batches per core
CH_TILES = 2                               # row-tiles per small DMA chunk
CH_ROWS = CH_TILES * TILE_ROWS             # 1024
ST_TILES = 8                               # row-tiles per bulk DMA chunk
ST_ROWS = ST_TILES * TILE_ROWS             # 4096
N_S = 4                                    # small chunks at stream start
N_ST_FULL = 5                              # bulk chunks
N_DR = 8                                   # small chunks at stream end
# tiles: 4*2 + 5*8 + 8*2 = 64

_CACHE = {}


def _build():
    f32 = mybir.dt.float32
    bf16 = mybir.dt.bfloat16
    fp8 = mybir.dt.float8e4
    AF = mybir.ActivationFunctionType
    OP = mybir.AluOpType
    PM = mybir.MatmulPerfMode

    nc = bacc.Bacc("TRN2", target_bir_lowering=False, debug=False,
                   enable_asserts=False, num_devices=N_CORES)
    # chunk-local layout: [chunk, p, a, b, rows] -> per partition the whole
    # (a, b, rows) block is one contiguous DMA descriptor (16KB bulk / 4KB
    # small)
    xS_d = nc.dram_tensor("xS", [N_S, 128, 2, 2, CH_ROWS], fp8,
                          kind="ExternalInput").ap()
    xA_d = nc.dram_tensor("xA", [N_ST_FULL, 128, 2, 2, ST_ROWS], fp8,
                          kind="ExternalInput").ap()
    xB_d = nc.dram_tensor("xB", [N_DR, 128, 2, 2, CH_ROWS], fp8,
                          kind="ExternalInput").ap()
    probs_d = nc.dram_tensor("probs", [NB, 128, G * 4], f32,
                             kind="ExternalInput").ap()
    w13_d = nc.dram_tensor("w13", [D, 128], fp8, kind="ExternalInput").ap()
    wcat_d = nc.dram_tensor("wcat", [128, 8], bf16, kind="ExternalInput").ap()
    b13_d = nc.dram_tensor("b13", [128, 1], f32, kind="ExternalInput").ap()
    b24_d = nc.dram_tensor("b24", [128, 512], f32, kind="ExternalInput").ap()
    out_d = nc.dram_tensor("out", [NB, 128, G * 2], f32,
                           kind="ExternalOutput").ap()

    with tile.TileContext(nc) as tc, ExitStack() as ctx:
        wpool = ctx.enter_context(tc.tile_pool(name="w", bufs=1))
        xsp = ctx.enter_context(tc.tile_pool(name="xp", bufs=N_S))
        xstp = ctx.enter_context(tc.tile_pool(name="xs", bufs=N_ST_FULL))
        xdp = ctx.enter_context(tc.tile_pool(name="xd", bufs=N_DR))
        hp = ctx.enter_context(tc.tile_pool(name="h", bufs=3))
        epool = ctx.enter_context(tc.tile_pool(name="e", bufs=2))
        ppool = ctx.enter_context(tc.tile_pool(name="p", bufs=NB))
        opool = ctx.enter_context(tc.tile_pool(name="o", bufs=2))
        tpool = ctx.enter_context(tc.tile_pool(name="t", bufs=2))
        pm1p = ctx.enter_context(tc.tile_pool(name="pm1", bufs=3, space="PSUM"))
        pm2p = ctx.enter_context(tc.tile_pool(name="pm2", bufs=2, space="PSUM"))

        # feature f = a*256 + b*128 + p  ->  [p, a(kk), b(d), .]
        # weights go on the Pool queue so the x stream starts immediately
        w13 = wpool.tile([128, 2, 2, 128], fp8, name="w13sb")
        nc.gpsimd.dma_start(w13[:], w13_d.rearrange("(a b p) h -> p a b h",
                                                    a=2, b=2))
        wcat = wpool.tile([128, 8], bf16, name="wcatsb")
        nc.gpsimd.dma_start(wcat[:], wcat_d)
        b13 = wpool.tile([128, 1], f32, name="b13sb")
        nc.gpsimd.dma_start(b13[:], b13_d)
        b24 = wpool.tile([128, 512], f32, name="b24sb")
        nc.gpsimd.dma_start(b24[:], b24_d)

        # ---- pre-issue every x/probs DMA (dedicated buffers) ----
        # probs first on the otherwise-empty Pool queue; all of x streams
        # on the SP HWDGE queue with no semaphore waits in between.
        prbufs = []
        for bt in range(NB):
            pr = ppool.tile([128, G, 4], f32, name="pr")
            nc.gpsimd.dma_start(
                pr[:], probs_d[bt].rearrange("q (g i) -> q g i", i=4))
            prbufs.append(pr)
        xbuf = []   # per tile: (tile_handle, col_offset)
        for ch in range(N_S):
            xt = xsp.tile([128, 2, 2, CH_ROWS], fp8, name="xsc")
            nc.sync.dma_start(xt[:], xS_d[ch])
            for j in range(CH_TILES):
                xbuf.append((xt, j * TILE_ROWS))
        for st in range(N_ST_FULL):
            xt = xstp.tile([128, 2, 2, ST_ROWS], fp8, name="xst")
            nc.sync.dma_start(xt[:], xA_d[st])
            for j in range(ST_TILES):
                xbuf.append((xt, j * TILE_ROWS))
        for ch in range(N_DR):
            xt = xdp.tile([128, 2, 2, CH_ROWS], fp8, name="xch")
            nc.sync.dma_start(xt[:], xB_d[ch])
            for j in range(CH_TILES):
                xbuf.append((xt, j * TILE_ROWS))

        def st_mm1(ti):
            # paired tiles share a 2-bank PSUM tile; even tile allocates
            t = ti["t"]
            if t % 2 == 0:
                ti["pm1b"] = pm1p.tile([128, 1024], f32, name="pm1b")
            pm1b = ti["pm1b"]
            half = (t % 2) * 512
            xt, co = xbuf[t]
            for kk in range(2):
                nc.tensor.matmul(pm1b[:, half:half + 512], w13[:, kk],
                                 xt[:, kk, :, co:co + 512],
                                 start=(kk == 0), stop=(kk == 1),
                                 perf_mode=PM.DoubleRow)

        def st_relu(ti):
            # fires on odd tiles: relu over the completed pair [128, 1024]
            H = hp.tile([128, 1024], bf16, name="H")
            nc.scalar.activation(H[:], ti["pm1b"][:], AF.Relu, bias=b13[:])
            ti["H"] = H

        def st_mm2(ti):
            t16 = ti["t16"]
            if t16 == 0:
                ti["pm2b"] = pm2p.tile([128, 512], f32, name="pm2b")
            pm2b = ti["pm2b"]
            H = ti["H"]
            hoff = (ti["t"] % 2) * 512
            for g in range(4):
                nc.tensor.matmul(pm2b[:, t16 * 32 + g * 8:t16 * 32 + g * 8 + 8],
                                 H[:, hoff + g * 128:hoff + (g + 1) * 128],
                                 wcat[:], start=True, stop=True)

        def st_esl(ti, half):
            # half-batch: E[:, c0:c1] = sigmoid(pm2b[, c0:c1] + b24), epilogue
            c0, c1 = half * 256, half * 256 + 256
            if half == 0:
                ti["E"] = epool.tile([128, 512], f32, name="E")
                ti["ob"] = opool.tile([128, G * 2], f32, name="ob")
            E = ti["E"]
            nc.vector.tensor_tensor(E[:, c0:c1], ti["pm2b"][:, c0:c1],
                                    b24[:, c0:c1], OP.add)
            nc.scalar.activation(E[:, c0:c1], E[:, c0:c1], AF.Sigmoid)
            epiq.append((E, prbufs[ti["bt"]], ti["bt"], ti["ob"], half))

        def do_epilogue_part(Eb, prb, ebt, ob, part):
            gs = slice(part * (G // 2), (part + 1) * (G // 2))
            GW = G // 2
            E3 = Eb.rearrange("q (g c) -> q g c", c=8)
            P3 = prb
            O3 = ob.rearrange("q (g k) -> q g k", k=2)
            mu1 = E3[:, gs, 0:2]
            mu2 = E3[:, gs, 2:4]
            inc = E3[:, gs, 4:6]
            p0 = P3[:, gs, 0:2]
            p1 = P3[:, gs, 2:4]
            mx = tpool.tile([128, GW, 2], f32, name="mx")
            nc.vector.tensor_tensor(mx[:], mu1, mu2, OP.max)
            nc.vector.tensor_tensor(mx[:], mx[:], inc, OP.add)
            nc.vector.tensor_scalar_min(mx[:], mx[:], 1.0)
            dm = tpool.tile([128, GW, 2], f32, name="dm")
            nc.vector.tensor_tensor(dm[:], p1, p0, OP.subtract)
            nc.vector.tensor_tensor(dm[:], dm[:], mx[:], OP.mult)
            r1 = tpool.tile([128, GW, 2], f32, name="r1")
            nc.vector.tensor_tensor(r1[:], p0, mu1, OP.mult)
            nc.vector.tensor_tensor(r1[:], r1[:], dm[:], OP.add)
            r2 = tpool.tile([128, GW, 2], f32, name="r2")
            nc.vector.tensor_tensor(r2[:], p1, mu2, OP.mult)
            nc.vector.tensor_tensor(r2[:], r2[:], dm[:], OP.subtract)
            msk = tpool.tile([128, GW, 2], mybir.dt.uint8, name="msk")
            nc.vector.tensor_tensor(msk[:], p0, p1, OP.is_le)
            nc.vector.copy_predicated(r2[:], msk[:], r1[:])
            dd = tpool.tile([128, GW], f32, name="dd")
            nc.vector.tensor_tensor(dd[:], r2[:, :, 0], r2[:, :, 1],
                                    OP.subtract)
            nc.scalar.activation(O3[:, gs, 0], dd[:], AF.Sigmoid)
            nc.vector.tensor_scalar(O3[:, gs, 1], O3[:, gs, 0], -1.0, 1.0,
                                    OP.mult, OP.add)
            if part == 1:
                nc.gpsimd.dma_start(out_d[ebt], ob[:])

        pm1b = None
        pm2b = None
        epiq = []
        esl_ctx = {}
        tiles = []
        for t in range(N_TILES + 4):
            if t < N_TILES:
                tiles.append({"t": t, "bt": t // TILES_PER_BATCH,
                              "t16": t % TILES_PER_BATCH})

            if t - 1 >= 0 and t - 1 < N_TILES:
                ti = tiles[t - 1]
                if ti["t"] % 2 == 1:
                    ti["pm1b"] = pm1b
                st_mm1(ti)
                pm1b = ti["pm1b"]
            if t - 2 >= 1 and t - 2 < N_TILES and (t - 2) % 2 == 1:
                ti = tiles[t - 2]
                st_relu(ti)
                tiles[t - 3]["H"] = ti["H"]
            if t - 3 >= 0 and t - 3 < N_TILES:
                ti = tiles[t - 3]
                if ti["t16"] != 0:
                    ti["pm2b"] = pm2b
                st_mm2(ti)
                pm2b = ti["pm2b"]
            if t - 4 >= 0 and t - 4 < N_TILES:
                ti = tiles[t - 4]
                if ti["t16"] == 7:
                    st_esl(ti, 0)
                    esl_ctx[ti["bt"]] = ti
                elif ti["t16"] == 15:
                    ti2 = esl_ctx.pop(ti["bt"])
                    ti["E"] = ti2["E"]
                    ti["ob"] = ti2["ob"]
                    st_esl(ti, 1)
            if epiq:
                do_epilogue_part(*epiq.pop(0))
        while epiq:
            do_epilogue_part(*epiq.pop(0))

    nc.compile()
    return nc


def _get_nc():
    if "nc" not in _CACHE:
        _CACHE["nc"] = _build()
    return _CACHE["nc"]


def _prep_host(probs, fuzzy_features, W1, b1, W2, b2, W3, b3, W4, b4):
    x = np.asarray(fuzzy_features, dtype=np.float32)
    pr = np.ascontiguousarray(np.asarray(probs, dtype=np.float32).reshape(B, 4))
    W1 = np.asarray(W1, np.float32); b1 = np.asarray(b1, np.float32)
    W2 = np.asarray(W2, np.float32); b2 = np.asarray(b2, np.float32)
    W3 = np.asarray(W3, np.float32); b3 = np.asarray(b3, np.float32)
    W4 = np.asarray(W4, np.float32); b4 = np.asarray(b4, np.float32)

    # per-core chunk-local transposed fp8 features:
    # feature f = a*256 + b*128 + p  ->  [chunk, p, a, b, rows]
    x8 = x.reshape(N_CORES, R, D).astype(ml_dtypes.float8_e4m3)
    rs = N_S * CH_ROWS
    ra = rs + N_ST_FULL * ST_ROWS
    xS = np.ascontiguousarray(
        x8[:, :rs].reshape(N_CORES, N_S, CH_ROWS, 2, 2, 128)
        .transpose(0, 1, 5, 3, 4, 2))
    xA = np.ascontiguousarray(
        x8[:, rs:ra].reshape(N_CORES, N_ST_FULL, ST_ROWS, 2, 2, 128)
        .transpose(0, 1, 5, 3, 4, 2))
    xB = np.ascontiguousarray(
        x8[:, ra:].reshape(N_CORES, N_DR, CH_ROWS, 2, 2, 128)
        .transpose(0, 1, 5, 3, 4, 2))

    # probs permuted so device row gg*128+q of batch bt is at [c, bt, q, gg]:
    prp = np.ascontiguousarray(
        pr.reshape(N_CORES, NB, G, 128, 4).transpose(0, 1, 3, 2, 4)
    ).reshape(N_CORES, NB, 128, G * 4)

    w13 = np.ascontiguousarray(
        np.concatenate([W1, W3], axis=1)).astype(ml_dtypes.float8_e4m3)
    wcat = np.zeros((128, 8), np.float32)
    wcat[0:64, 0:4] = W2
    wcat[64:128, 4:6] = W4
    wcat = wcat.astype(ml_dtypes.bfloat16)
    b13 = np.concatenate([b1, b3]).reshape(128, 1)
    pat = np.concatenate([b2, b4, np.zeros(2, np.float32)])             # [8]
    b24 = np.ascontiguousarray(np.tile(pat, (128, 64)))                 # [128, 512]

    in_maps = []
    for c in range(N_CORES):
        in_maps.append({
            "xS": xS[c], "xA": xA[c], "xB": xB[c],
            "probs": prp[c],
            "w13": w13, "wcat": wcat, "b13": b13, "b24": b24,
        })
    return in_maps


def _unpermute_out(res):
    # device out: [c][bt, q, gg*2] with row gg*128+q -> [B, 2]
    outs = []
    for c in range(N_CORES):
        o = res.results[c]["out"].reshape(NB, 128, G, 2)
        outs.append(o.transpose(0, 2, 1, 3).reshape(R, 2))
    return np.concatenate(outs, axis=0)


def kernel(probs, fuzzy_features, W1, b1, W2, b2, W3, b3, W4, b4, **kwargs):
    nc = _get_nc()
    in_maps = _prep_host(probs, fuzzy_features, W1, b1, W2, b2, W3, b3, W4, b4)
    res = bass_utils.run_bass_kernel_spmd(nc, in_maps, core_ids=list(range(N_CORES)))
    return _unpermute_out(res)


# revision 64
# speedup vs baseline: 1.0365x; 1.0365x over previous
"""Trainium2 Bass kernel for nn_MF2Net (two tiny MLPs + Choquet integral + softmax).

Strategy: pure data parallel over the batch dim (8 NeuronCores x 32768 rows).
x is shipped host-transposed as fp8-e4m3 [512, R] so features land on SBUF
partitions straight from DMA (no on-chip transpose, 1/4 the f32 HBM traffic).
Per core:
  - x^T DMAs all pre-issued across two queues (SP HWDGE + Pool SWDGE):
    6 supertiles of 4096 rows, then 16 per-tile chunks for a short drain
  - matmul1 per 512-row tile: H^T[128hid, 512] = DoubleRow-fp8 matmul
    (K=512 as 2 packed 256-chunks) into paired 2-bank f32 PSUM
  - ACT: H = relu(H^T + b13) per tile-pair [128, 1024] -> bf16
  - matmul2 (bf16): S[128 rows, 8] per 128-row group into a per-batch PSUM
    bank [128, 512]; lhsT = H chunk, rhs = Wcat ([W2|W4] block-diag)
  - per half-batch: DVE bias add + ACT sigmoid on [128, 256], then
    Choquet combine (both classes fused) + softmax via sigmoid(res0-res1)
Row mapping on device is purely sequential; probs/out are host-permuted to
match (cheap: 0.5 MB + 0.25 MB per core vs 64 MB for x).
"""
import numpy as np
import ml_dtypes
from contextlib import ExitStack

import concourse.bass as bass
import concourse.bacc as bacc
import concourse.tile as tile
import concourse.mybir as mybir
from concourse import bass_utils

N_CORES = 8
B = 262144
D = 512
R = B // N_CORES            # rows per core
TILE_ROWS = 512
N_TILES = R // TILE_ROWS    # 64
TILES_PER_BATCH = 16
BATCH_ROWS = TILE_ROWS * TILES_PER_BATCH   # 8192
G = BATCH_ROWS // 128                      # 64 row-groups per batch
NB = R // BATCH_ROWS                       # 4 batches per core
CH_TILES = 2                               # row-tiles per small DMA chunk
CH_ROWS = CH_TILES * TILE_ROWS             # 1024
ST_TILES = 8                               # row-tiles per bulk DMA chunk
ST_ROWS = ST_TILES * TILE_ROWS             # 4096
N_S = 0                                    # small chunks at stream start
N_ST_FULL = 6                              # bulk chunks
N_DR = 8                                   # small chunks at stream end
# tiles: 6*8 + 8*2 = 64

_CACHE = {}


def _build():
    f32 = mybir.dt.float32
    bf16 = mybir.dt.bfloat16
    fp8 = mybir.dt.float8e4
    AF = mybir.ActivationFunctionType
    OP = mybir.AluOpType
    PM = mybir.MatmulPerfMode

    nc = bacc.Bacc("TRN2", target_bir_lowering=False, debug=False,
                   enable_asserts=False, num_devices=N_CORES)
    # chunk-local layout: [chunk, p, a, b, rows] -> per partition the whole
    # (a, b, rows) block is one contiguous DMA descriptor (16KB bulk / 4KB
    # small)
    xA_d = nc.dram_tensor("xA", [N_ST_FULL, 128, 2, 2, ST_ROWS], fp8,
                          kind="ExternalInput").ap()
    xB_d = nc.dram_tensor("xB", [N_DR, 128, 2, 2, CH_ROWS], fp8,
                          kind="ExternalInput").ap()
    probs_d = nc.dram_tensor("probs", [NB, 128, G * 4], f32,
                             kind="ExternalInput").ap()
    w13_d = nc.dram_tensor("w13", [D, 128], fp8, kind="ExternalInput").ap()
    wcat_d = nc.dram_tensor("wcat", [128, 8], bf16, kind="ExternalInput").ap()
    b13_d = nc.dram_tensor("b13", [128, 1], f32, kind="ExternalInput").ap()
    b24_d = nc.dram_tensor("b24", [128, 512], f32, kind="ExternalInput").ap()
    out_d = nc.dram_tensor("out", [NB, 128, G * 2], f32,
                           kind="ExternalOutput").ap()

    with tile.TileContext(nc) as tc, ExitStack() as ctx:
        wpool = ctx.enter_context(tc.tile_pool(name="w", bufs=1))
        xstp = ctx.enter_context(tc.tile_pool(name="xs", bufs=N_ST_FULL))
        xdp = ctx.enter_context(tc.tile_pool(name="xd", bufs=N_DR))
        hp = ctx.enter_context(tc.tile_pool(name="h", bufs=3))
        epool = ctx.enter_context(tc.tile_pool(name="e", bufs=2))
        ppool = ctx.enter_context(tc.tile_pool(name="p", bufs=NB))
        opool = ctx.enter_context(tc.tile_pool(name="o", bufs=2))
        tpool = ctx.enter_context(tc.tile_pool(name="t", bufs=2))
        pm1p = ctx.enter_context(tc.tile_pool(name="pm1", bufs=3, space="PSUM"))
        pm2p = ctx.enter_context(tc.tile_pool(name="pm2", bufs=2, space="PSUM"))

        # feature f = a*256 + b*128 + p  ->  [p, a(kk), b(d), .]
        # weights go on the Pool queue so the x stream starts immediately
        w13 = wpool.tile([128, 2, 2, 128], fp8, name="w13sb")
        nc.gpsimd.dma_start(w13[:], w13_d.rearrange("(a b p) h -> p a b h",
                                                    a=2, b=2))
        wcat = wpool.tile([128, 8], bf16, name="wcatsb")
        nc.gpsimd.dma_start(wcat[:], wcat_d)
        b13 = wpool.tile([128, 1], f32, name="b13sb")
        nc.gpsimd.dma_start(b13[:], b13_d)
        b24 = wpool.tile([128, 512], f32, name="b24sb")
        nc.gpsimd.dma_start(b24[:], b24_d)

        # ---- pre-issue every x/probs DMA (dedicated buffers) ----
        # probs first on the otherwise-empty Pool queue; all of x streams
        # on the SP HWDGE queue with no semaphore waits in between.
        prbufs = []
        for bt in range(NB):
            pr = ppool.tile([128, G, 4], f32, name="pr")
            nc.gpsimd.dma_start(
                pr[:], probs_d[bt].rearrange("q (g i) -> q g i", i=4))
            prbufs.append(pr)
        xbuf = []   # per tile: (tile_handle, col_offset)
        for st in range(N_ST_FULL):
            xt = xstp.tile([128, 2, 2, ST_ROWS], fp8, name="xst")
            nc.sync.dma_start(xt[:], xA_d[st])
            for j in range(ST_TILES):
                xbuf.append((xt, j * TILE_ROWS))
        for ch in range(N_DR):
            xt = xdp.tile([128, 2, 2, CH_ROWS], fp8, name="xch")
            nc.sync.dma_start(xt[:], xB_d[ch])
            for j in range(CH_TILES):
                xbuf.append((xt, j * TILE_ROWS))

        def st_mm1(ti):
            # paired tiles share a 2-bank PSUM tile; even tile allocates
            t = ti["t"]
            if t % 2 == 0:
                ti["pm1b"] = pm1p.tile([128, 1024], f32, name="pm1b")
            pm1b = ti["pm1b"]
            half = (t % 2) * 512
            xt, co = xbuf[t]
            for kk in range(2):
                nc.tensor.matmul(pm1b[:, half:half + 512], w13[:, kk],
                                 xt[:, kk, :, co:co + 512],
                                 start=(kk == 0), stop=(kk == 1),
                                 perf_mode=PM.DoubleRow)

        def st_relu(ti):
            # fires on odd tiles: relu over the completed pair [128, 1024]
            H = hp.tile([128, 1024], bf16, name="H")
            nc.scalar.activation(H[:], ti["pm1b"][:], AF.Relu, bias=b13[:])
            ti["H"] = H

        def st_mm2(ti):
            t16 = ti["t16"]
            if t16 == 0:
                ti["pm2b"] = pm2p.tile([128, 512], f32, name="pm2b")
            pm2b = ti["pm2b"]
            H = ti["H"]
            hoff = (ti["t"] % 2) * 512
            for g in range(4):
                nc.tensor.matmul(pm2b[:, t16 * 32 + g * 8:t16 * 32 + g * 8 + 8],
                                 H[:, hoff + g * 128:hoff + (g + 1) * 128],
                                 wcat[:], start=True, stop=True)

        def st_esl(ti, half):
            # half-batch: E[:, c0:c1] = sigmoid(pm2b[, c0:c1] + b24), epilogue
            c0, c1 = half * 256, half * 256 + 256
            if half == 0:
                ti["E"] = epool.tile([128, 512], f32, name="E")
                ti["ob"] = opool.tile([128, G * 2], f32, name="ob")
            E = ti["E"]
            nc.vector.tensor_tensor(E[:, c0:c1], ti["pm2b"][:, c0:c1],
                                    b24[:, c0:c1], OP.add)
            nc.scalar.activation(E[:, c0:c1], E[:, c0:c1], AF.Sigmoid)
            epiq.append((E, prbufs[ti["bt"]], ti["bt"], ti["ob"], half))

        def do_epilogue_part(Eb, prb, ebt, ob, part):
            gs = slice(part * (G // 2), (part + 1) * (G // 2))
            GW = G // 2
            E3 = Eb.rearrange("q (g c) -> q g c", c=8)
            P3 = prb
            O3 = ob.rearrange("q (g k) -> q g k", k=2)
            mu1 = E3[:, gs, 0:2]
            mu2 = E3[:, gs, 2:4]
            inc = E3[:, gs, 4:6]
            p0 = P3[:, gs, 0:2]
            p1 = P3[:, gs, 2:4]
            mx = tpool.tile([128, GW, 2], f32, name="mx")
            nc.vector.tensor_tensor(mx[:], mu1, mu2, OP.max)
            nc.vector.tensor_tensor(mx[:], mx[:], inc, OP.add)
            nc.vector.tensor_scalar_min(mx[:], mx[:], 1.0)
            dm = tpool.tile([128, GW, 2], f32, name="dm")
            nc.vector.tensor_tensor(dm[:], p1, p0, OP.subtract)
            nc.vector.tensor_tensor(dm[:], dm[:], mx[:], OP.mult)
            r1 = tpool.tile([128, GW, 2], f32, name="r1")
            nc.vector.tensor_tensor(r1[:], p0, mu1, OP.mult)
            nc.vector.tensor_tensor(r1[:], r1[:], dm[:], OP.add)
            r2 = tpool.tile([128, GW, 2], f32, name="r2")
            nc.vector.tensor_tensor(r2[:], p1, mu2, OP.mult)
            nc.vector.tensor_tensor(r2[:], r2[:], dm[:], OP.subtract)
            msk = tpool.tile([128, GW, 2], mybir.dt.uint8, name="msk")
            nc.vector.tensor_tensor(msk[:], p0, p1, OP.is_le)
            nc.vector.copy_predicated(r2[:], msk[:], r1[:])
            dd = tpool.tile([128, GW], f32, name="dd")
            nc.vector.tensor_tensor(dd[:], r2[:, :, 0], r2[:, :, 1],
                                    OP.subtract)
            nc.scalar.activation(O3[:, gs, 0], dd[:], AF.Sigmoid)
            nc.vector.tensor_scalar(O3[:, gs, 1], O3[:, gs, 0], -1.0, 1.0,
                                    OP.mult, OP.add)
            if part == 1:
                nc.gpsimd.dma_start(out_d[ebt], ob[:])

        pm1b = None
        pm2b = None
        epiq = []
        esl_ctx = {}
        tiles = []
        for t in range(N_TILES + 4):
            if t < N_TILES:
                tiles.append({"t": t, "bt": t // TILES_PER_BATCH,
                              "t16": t % TILES_PER_BATCH})

            if t - 1 >= 0 and t - 1 < N_TILES:
                ti = tiles[t - 1]
                if ti["t"] % 2 == 1:
                    ti["pm1b"] = pm1b
                st_mm1(ti)
                pm1b = ti["pm1b"]
            if t - 2 >= 1 and t - 2 < N_TILES and (t - 2) % 2 == 1:
                ti = tiles[t - 2]
                st_relu(ti)
                tiles[t - 3]["H"] = ti["H"]
            if t - 3 >= 0 and t - 3 < N_TILES:
                ti = tiles[t - 3]
                if ti["t16"] != 0:
                    ti["pm2b"] = pm2b
                st_mm2(ti)
                pm2b = ti["pm2b"]
            if t - 4 >= 0 and t - 4 < N_TILES:
                ti = tiles[t - 4]
                if ti["t16"] == 7:
                    st_esl(ti, 0)
                    esl_ctx[ti["bt"]] = ti
                elif ti["t16"] == 15:
                    ti2 = esl_ctx.pop(ti["bt"])
                    ti["E"] = ti2["E"]
                    ti["ob"] = ti2["ob"]
                    st_esl(ti, 1)
            if epiq:
                do_epilogue_part(*epiq.pop(0))
        while epiq:
            do_epilogue_part(*epiq.pop(0))

    nc.compile()
    return nc


def _get_nc():
    if "nc" not in _CACHE:
        _CACHE["nc"] = _build()
    return _CACHE["nc"]


def _prep_host(probs, fuzzy_features, W1, b1, W2, b2, W3, b3, W4, b4):
    x = np.asarray(fuzzy_features, dtype=np.float32)
    pr = np.ascontiguousarray(np.asarray(probs, dtype=np.float32).reshape(B, 4))
    W1 = np.asarray(W1, np.float32); b1 = np.asarray(b1, np.float32)
    W2 = np.asarray(W2, np.float32); b2 = np.asarray(b2, np.float32)
    W3 = np.asarray(W3, np.float32); b3 = np.asarray(b3, np.float32)
    W4 = np.asarray(W4, np.float32); b4 = np.asarray(b4, np.float32)

    # per-core chunk-local transposed fp8 features:
    # feature f = a*256 + b*128 + p  ->  [chunk, p, a, b, rows]
    x8 = x.reshape(N_CORES, R, D).astype(ml_dtypes.float8_e4m3)
    ra = N_ST_FULL * ST_ROWS
    xA = np.ascontiguousarray(
        x8[:, :ra].reshape(N_CORES, N_ST_FULL, ST_ROWS, 2, 2, 128)
        .transpose(0, 1, 5, 3, 4, 2))
    xB = np.ascontiguousarray(
        x8[:, ra:].reshape(N_CORES, N_DR, CH_ROWS, 2, 2, 128)
        .transpose(0, 1, 5, 3, 4, 2))

    # probs permuted so device row gg*128+q of batch bt is at [c, bt, q, gg]:
    prp = np.ascontiguousarray(
        pr.reshape(N_CORES, NB, G, 128, 4).transpose(0, 1, 3, 2, 4)
    ).reshape(N_CORES, NB, 128, G * 4)

    w13 = np.ascontiguousarray(
        np.concatenate([W1, W3], axis=1)).astype(ml_dtypes.float8_e4m3)
    wcat = np.zeros((128, 8), np.float32)
    wcat[0:64, 0:4] = W2
    wcat[64:128, 4:6] = W4
    wcat = wcat.astype(ml_dtypes.bfloat16)
    b13 = np.concatenate([b1, b3]).reshape(128, 1)
    pat = np.concatenate([b2, b4, np.zeros(2, np.float32)])             # [8]
    b24 = np.ascontiguousarray(np.tile(pat, (128, 64)))                 # [128, 512]

    in_maps = []
    for c in range(N_CORES):
        in_maps.append({
            "xA": xA[c], "xB": xB[c],
            "probs": prp[c],
            "w13": w13, "wcat": wcat, "b13": b13, "b24": b24,
        })
    return in_maps


def _unpermute_out(res):
    # device out: [c][bt, q, gg*2] with row gg*128+q -> [B, 2]
    outs = []
    for c in range(N_CORES):
        o = res.results[c]["out"].reshape(NB, 128, G, 2)
        outs.append(o.transpose(0, 2, 1, 3).reshape(R, 2))
    return np.concatenate(outs, axis=0)


def kernel(probs, fuzzy_features, W1, b1, W2, b2, W3, b3, W4, b4, **kwargs):
    nc = _get_nc()
    in_maps = _prep_host(probs, fuzzy_features, W1, b1, W2, b2, W3, b3, W4, b4)
    res = bass_utils.run_bass_kernel_spmd(nc, in_maps, core_ids=list(range(N_CORES)))
    return _unpermute_out(res)
